# revision 22
# baseline (speedup 1.0000x reference)
"""Trainium2 Bass kernel for the sparse-attention (local 3x3 unfold) problem.

Math (per batch-channel (b,c), H=W=128, K=3, pad=1):
  ku = unfold(key)  -> [9, L] raw-flat, reinterpreted [L, 9]
  qu = unfold(query)
  out1 = ku * qu[:, 4:5] ; out2 = ku[:, 4:5] * qu   (as [L, 9] views)

The flat per-channel output index n in [0, 9L) decomposes two ways:
  * n = 128*q + j           (chunk q = one (patch p2=q//128, row i2=q%128)
                             slice: 128 contiguous floats of a dj-shifted,
                             row-padded image variant)
  * n = 9*g + e             (group g shares one stride-9 "center" factor)

Device layout (v2, "fat rows"): channel ch of a tile owns 16 partitions
(r = 16*ch + rr) with FREE = 9216 = 72 chunks per partition, n = 9216*rr + f.
  * FREE % 9 == 0 keeps the stride-9 center-broadcast multiply phase-free
    on every partition (one DVE op covers all 8 channels of a tile).
  * Loads: the (72-chunk partition) x (128-chunk patch) overlap gives 24
    maximal segments per channel; each is ONE contiguous DRAM run of the
    variant image -> one descriptor (2-18 KiB) per (segment, channel).
  * Stores: per-channel DRAM is contiguous with offset r*FREE uniform in
    the partition index -> one dma_start moves a whole tile half
    (128 descriptors x 9 KiB).

dtype: fp16 end-to-end on device (harness tolerance 2e-2 vs ~1.5e-3 fp16
error); host upcasts to fp32.  Halves both HBM read and write traffic.

Sharding: pure data-parallel over the 256 (b,c) channels; 32 per core.
"""

import sys

for _p in ("/opt/trn_rl_repo", "/opt/pypackages"):
    if _p not in sys.path:
        sys.path.insert(0, _p)

import numpy as np

import concourse.bass as bass
import concourse.mybir as mybir
import concourse.tile as tile
from concourse.bass import AP
from concourse.bass_utils import run_bass_kernel_spmd
from concourse.vector_clock import ScopedClock

# ---------------------------------------------------------------------------
# Patch: this container's walrus rejects >1 sync-wait on the Tile tail Drain
# ("Too many sync wait commands").  Spill extra waits onto SP NOPs, which
# execute in program order before the all-engine barrier, preserving the
# "all work done before sem clear" semantics.
# ---------------------------------------------------------------------------


def _drain_and_barrier(self, tick_clock, wait_clock):
    nc = self.nc
    drain_inst = nc.sync.drain()
    wait_clock.add_sem_waits(
        drain_inst.ins, ScopedClock({None: tick_clock.global_clock})
    )
    si = drain_inst.ins.sync_info
    if si is not None and len(si.on_wait) > 1:
        waits = list(si.on_wait)
        drain_inst.ins.sync_info = mybir.SyncInfo(
            on_wait=waits[:1], on_update=list(si.on_update)
        )
        for w in waits[1:]:
            nop = nc.sync.nop(nofuse=True)
            nop.ins.sync_info = mybir.SyncInfo(on_wait=[w], on_update=[])

    nc.all_engine_barrier()
    assert self.sems is not None
    popped = nc._tile_sem_poison_stack.pop()
    assert popped is self._sem_poison
    nc.clear_and_free_semaphores(list(self.sems.allocated().values()))
    nc.all_engine_barrier()


tile.TileContext._drain_and_barrier = _drain_and_barrier


def _split_waits(nc, maxw=1):
    """Walrus here allows only `maxw` sync-waits per instruction: move extra
    waits onto same-engine NOPs inserted immediately before the instruction
    (same engine stream => executes before it)."""
    for fn in nc.m.functions:
        for bb in fn.blocks:
            out = []
            for inst in bb.instructions:
                si = getattr(inst, "sync_info", None)
                if si is not None and len(si.on_wait) > maxw:
                    waits = list(si.on_wait)
                    for w in waits[:-maxw]:
                        nop = mybir.InstNoOp(
                            name=nc.get_next_instruction_name(),
                            bass_nofuse=True,
                        )
                        nop.engine = inst.engine
                        nop.sync_info = mybir.SyncInfo(on_wait=[w], on_update=[])
                        nc.register_instruction(nop)
                        out.append(nop)
                    inst.sync_info = mybir.SyncInfo(
                        on_wait=waits[-maxw:], on_update=list(si.on_update)
                    )
                out.append(inst)
            bb.instructions[:] = out

# ---------------------------------------------------------------------------

F16 = mybir.dt.float16

N_CORES = 8
B, C, H, W = 4, 64, 128, 128
BC = B * C                # 256 channels
CPC = BC // N_CORES       # 32 channels per core
NCH = 16                  # channels per tile (x8 partitions = 128)
NG = CPC // NCH           # channel groups per core
HP = H + 2                # padded rows
VAR = HP * W              # one dj-variant: [130, 128]
IMG = 3 * VAR             # three dj-variants per channel
L = H * W
PPCH = 8                  # partitions per channel
CHF = 18432               # elements per partition per channel (144 chunks)
OUT_CH = 9 * L            # 147456 = PPCH * CHF

# Per-group sub-tile chunk widths (must each be a multiple of 9 so the
# stride-9 multiply stays phase-free, and sum to 144).  Tapered: small
# sub-tiles at the pipeline head (first mul starts sooner) and tail
# (short drain after the last loads).
SCHED = [
    [18, 18, 18, 18, 18, 18, 18, 18],
    [18, 18, 18, 18, 18, 18, 18, 18],
]
assert all(sum(s) == 144 and all(w * 128 % 9 == 0 for w in s) for s in SCHED)
MAXW = max(w for s in SCHED for w in s)
MAXF = MAXW * 128         # largest sub-tile free width per fused half


def _segments(sched):
    """Per sub-tile: (FREE, f_base, [(rr, f_off, len, src_off), ...]) --
    maximal q-runs inside the sub-tile window not crossing partition
    (144) or patch (128) boundaries.  f_base = chunk offset within CHF."""
    starts = [0]
    for w in sched:
        starts.append(starts[-1] + w)
    tiles = []
    for t, w in enumerate(sched):
        s0 = starts[t]
        bounds = sorted(
            {q for q in range(0, 1153, 144)}
            | {q for q in range(0, 1153, 128)}
            | {144 * r + s for r in range(8) for s in (s0, s0 + w)}
        )
        segs = []
        for qs, qe in zip(bounds[:-1], bounds[1:]):
            rr = qs // 144
            s = qs - 144 * rr
            if not (s0 <= s < s0 + w):
                continue
            p2 = qs // 128
            di, dj = divmod(p2, 3)
            segs.append(
                (rr, (s - s0) * 128, (qe - qs) * 128,
                 dj * VAR + (qs - 128 * p2 + di) * W)
            )
        tiles.append((w * 128, s0 * 128, segs))
    return tiles


_TILES = [_segments(s) for s in SCHED]


def _build_program():
    nc = bass.Bass(trn_type="TRN2")
    # k and q fused on a leading [2] axis (and o1/o2 likewise) so one
    # dma_start covers both: 32-descriptor loads / 256-descriptor stores
    # halve the dma_start count and keep all 16 SDMA engines 2 deep.
    kq = nc.dram_tensor("kq", [2, CPC, 3, HP, W], F16, kind="ExternalInput")
    oo = nc.dram_tensor("oo", [2, CPC, OUT_CH], F16, kind="ExternalOutput")
    IN_X = CPC * IMG          # DRAM stride between k and q planes
    OUT_X = CPC * OUT_CH      # DRAM stride between o1 and o2 planes

    # Three dynamic DMA queues (SP-HWDGE, ACT-HWDGE, Pool-SWDGE); strict
    # round-robin keeps every queue fed (prior HW finding: greedy
    # bin-packing clusters DMAs per queue and the per-engine FIFO then
    # serializes them).
    engines = [nc.sync, nc.scalar, nc.gpsimd]
    eng_i = [0]

    def eng():
        e = engines[eng_i[0] % len(engines)]
        eng_i[0] += 1
        return e

    MF2 = 2 * MAXF  # allocated fused tile free width: k then q (o1 then o2)

    def do_loads(g, free, segs, ti):
        # 32 descriptors (channel-major x {k,q}) per dma_start, mutually
        # non-contiguous in stream order.  Descriptors are dealt to
        # SDMA-engine slots round-robin from slot 0 and consecutive
        # contiguous descriptors re-aggregate into one packet, so
        # 8-descriptor loads pile onto engines 0-7 (HW-measured: 86%
        # busy vs 39% on engines 8-15); 32 descriptors keep all 16
        # engines 2 deep.
        th = ti[:].tensor
        for rr, f_off, seg_len, src_off in segs:
            eng().dma_start(
                AP(th, rr * MF2 + f_off,
                   [[PPCH * MF2, NCH], [free, 2], [1, seg_len]]),
                AP(kq, g * NCH * IMG + src_off,
                   [[IMG, NCH], [IN_X, 2], [1, seg_len]]),
            )

    def do_cen(free, ti, cen):
        # Expand the stride-9 centers into a DENSE tile (plane 0 =
        # q-centers for o1, plane 1 = k-centers for o2) on the otherwise
        # idle ACT/GpSimd engines.  The DVE multiplies then have all
        # operands packed step-1 fp16 -> 2x perf mode (HW packs two
        # 16-bit elements per 32-bit read port; the 0-stride broadcast
        # operand of the fused form forces 1x).
        ith, cenh = ti[:].tensor, cen[:].tensor
        n9 = free // 9
        nc.scalar.activation(
            AP(cenh, 0, [[MF2, 128], [9, n9], [1, 9]]),
            AP(ith, free + 4, [[MF2, 128], [9, n9], [0, 9]]),
            mybir.ActivationFunctionType.Copy,
        )
        nc.gpsimd.tensor_copy(
            AP(cenh, free, [[MF2, 128], [9, n9], [1, 9]]),
            AP(ith, 4, [[MF2, 128], [9, n9], [0, 9]]),
        )

    def do_mul_store(g, free, f_base, ti, cen, ot):
        ith, cenh, oth = ti[:].tensor, cen[:].tensor, ot[:].tensor
        ap = [[MF2, 128], [1, free]]
        # o1 = k_full * q_center ; o2 = q_full * k_center.  Each output's
        # store issues right after its own multiply so the second half of
        # the store traffic isn't gated on both muls (shorter drain).
        # DRAM per channel is contiguous: partition r = 8*ch + rr maps
        # to offset r*CHF + f_base, uniform across all 128 partitions.
        for x in (0, 1):
            nc.vector.tensor_mul(
                AP(oth, x * free, ap),
                AP(ith, x * free, ap),
                AP(cenh, x * free, ap),
            )
            eng().dma_start(
                AP(oo, x * OUT_X + g * NCH * OUT_CH + f_base,
                   [[CHF, 128], [1, free]]),
                AP(oth, x * free, [[MF2, 128], [1, free]]),
            )

    with tile.TileContext(nc) as tc:
        with (
            tc.tile_pool(name="tin", bufs=4) as tin,
            tc.tile_pool(name="tcen", bufs=3) as tcen,
            tc.tile_pool(name="tout", bufs=3) as tout,
        ):
            # Two-stage lookahead: loads(n) | center-copies(n-1) |
            # muls+stores(n-2).  DMA-queue FIFOs see all loads ahead of
            # the mul-gated stores, and the ACT/GpSimd center-copies
            # (which wait on loads n-1) never head-of-line-block the
            # dma_starts they issue for tile n.
            stage = []
            for g in range(NG):
                for free, f_base, segs in _TILES[g]:
                    ti = tin.tile([128, MF2], F16, tag="ti")
                    do_loads(g, free, segs, ti)
                    cen = tcen.tile([128, MF2], F16, tag="cen")
                    ot = tout.tile([128, MF2], F16, tag="ot")
                    stage.append((g, free, f_base, ti, cen, ot))
                    if len(stage) >= 2:
                        do_cen(stage[-2][1], stage[-2][3], stage[-2][4])
                    if len(stage) >= 3:
                        do_mul_store(*stage[-3])
            do_cen(stage[-1][1], stage[-1][3], stage[-1][4])
            do_mul_store(*stage[-2])
            do_mul_store(*stage[-1])
    _split_waits(nc)
    return nc


_NC_CACHE = []


def _get_nc():
    if not _NC_CACHE:
        _NC_CACHE.append(_build_program())
    return _NC_CACHE[0]


def _variants(x):
    """[B,C,H,W] -> [BC, 3, HP, W] fp16: dj-shifted, row-padded column
    windows of the zero-padded image."""
    xpad = np.pad(
        np.ascontiguousarray(x, dtype=np.float32).reshape(BC, H, W),
        ((0, 0), (1, 1), (1, 1)),
    )
    v = np.stack([xpad[:, :, j : j + W] for j in range(3)], axis=1)
    return np.ascontiguousarray(v.astype(np.float16))


def make_in_maps(key_map, query_map):
    kv = _variants(key_map)
    qv = _variants(query_map)
    return [
        {
            "kq": np.ascontiguousarray(
                np.stack([kv[m * CPC : (m + 1) * CPC],
                          qv[m * CPC : (m + 1) * CPC]])
            ),
        }
        for m in range(N_CORES)
    ]


def assemble(results):
    out1 = np.concatenate([results[m]["oo"][0] for m in range(N_CORES)], axis=0)
    out2 = np.concatenate([results[m]["oo"][1] for m in range(N_CORES)], axis=0)
    return (
        out1.reshape(B, C, L, 9).astype(np.float32),
        out2.reshape(B, C, L, 9).astype(np.float32),
    )


def kernel(key_map, query_map):
    nc = _get_nc()
    in_maps = make_in_maps(key_map, query_map)
    res = run_bass_kernel_spmd(nc, in_maps, core_ids=list(range(N_CORES)))
    return assemble(res.results)


# revision 24
# speedup vs baseline: 1.7888x; 1.7888x over previous
"""Trainium2 Bass kernel for the sparse-attention (local 3x3 unfold) problem.

Math (per batch-channel (b,c), H=W=128, K=3, pad=1):
  ku = unfold(key)  -> [9, L] raw-flat, reinterpreted [L, 9]
  qu = unfold(query)
  out1 = ku * qu[:, 4:5] ; out2 = ku[:, 4:5] * qu   (as [L, 9] views)

The flat per-channel output index n in [0, 9L) decomposes two ways:
  * n = 128*q + j           (chunk q = one (patch p2=q//128, row i2=q%128)
                             slice: 128 contiguous floats of a dj-shifted,
                             row-padded image variant)
  * n = 9*g + e             (group g shares one stride-9 "center" factor)

Device layout (v2, "fat rows"): channel ch of a tile owns 16 partitions
(r = 16*ch + rr) with FREE = 9216 = 72 chunks per partition, n = 9216*rr + f.
  * FREE % 9 == 0 keeps the stride-9 center-broadcast multiply phase-free
    on every partition (one DVE op covers all 8 channels of a tile).
  * Loads: the (72-chunk partition) x (128-chunk patch) overlap gives 24
    maximal segments per channel; each is ONE contiguous DRAM run of the
    variant image -> one descriptor (2-18 KiB) per (segment, channel).
  * Stores: per-channel DRAM is contiguous with offset r*FREE uniform in
    the partition index -> one dma_start moves a whole tile half
    (128 descriptors x 9 KiB).

dtype: fp16 end-to-end on device (harness tolerance 2e-2 vs ~1.5e-3 fp16
error); host upcasts to fp32.  Halves both HBM read and write traffic.

Sharding: pure data-parallel over the 256 (b,c) channels; 32 per core.
"""

import sys

for _p in ("/opt/trn_rl_repo", "/opt/pypackages"):
    if _p not in sys.path:
        sys.path.insert(0, _p)

import numpy as np

import concourse.bass as bass
import concourse.mybir as mybir
import concourse.tile as tile
from concourse.bass import AP
from concourse.bass_utils import run_bass_kernel_spmd
from concourse.vector_clock import ScopedClock

# ---------------------------------------------------------------------------
# Patch: this container's walrus rejects >1 sync-wait on the Tile tail Drain
# ("Too many sync wait commands").  Spill extra waits onto SP NOPs, which
# execute in program order before the all-engine barrier, preserving the
# "all work done before sem clear" semantics.
# ---------------------------------------------------------------------------


def _drain_and_barrier(self, tick_clock, wait_clock):
    nc = self.nc
    drain_inst = nc.sync.drain()
    wait_clock.add_sem_waits(
        drain_inst.ins, ScopedClock({None: tick_clock.global_clock})
    )
    si = drain_inst.ins.sync_info
    if si is not None and len(si.on_wait) > 1:
        waits = list(si.on_wait)
        drain_inst.ins.sync_info = mybir.SyncInfo(
            on_wait=waits[:1], on_update=list(si.on_update)
        )
        for w in waits[1:]:
            nop = nc.sync.nop(nofuse=True)
            nop.ins.sync_info = mybir.SyncInfo(on_wait=[w], on_update=[])

    nc.all_engine_barrier()
    assert self.sems is not None
    popped = nc._tile_sem_poison_stack.pop()
    assert popped is self._sem_poison
    nc.clear_and_free_semaphores(list(self.sems.allocated().values()))
    nc.all_engine_barrier()


tile.TileContext._drain_and_barrier = _drain_and_barrier


def _split_waits(nc, maxw=1):
    """Walrus here allows only `maxw` sync-waits per instruction: move extra
    waits onto same-engine NOPs inserted immediately before the instruction
    (same engine stream => executes before it)."""
    for fn in nc.m.functions:
        for bb in fn.blocks:
            out = []
            for inst in bb.instructions:
                si = getattr(inst, "sync_info", None)
                if si is not None and len(si.on_wait) > maxw:
                    waits = list(si.on_wait)
                    for w in waits[:-maxw]:
                        nop = mybir.InstNoOp(
                            name=nc.get_next_instruction_name(),
                            bass_nofuse=True,
                        )
                        nop.engine = inst.engine
                        nop.sync_info = mybir.SyncInfo(on_wait=[w], on_update=[])
                        nc.register_instruction(nop)
                        out.append(nop)
                    inst.sync_info = mybir.SyncInfo(
                        on_wait=waits[-maxw:], on_update=list(si.on_update)
                    )
                out.append(inst)
            bb.instructions[:] = out

# ---------------------------------------------------------------------------

F16 = mybir.dt.float16

N_CORES = 8
B, C, H, W = 4, 64, 128, 128
BC = B * C                # 256 channels
CPC = BC // N_CORES       # 32 channels per core
NCH = 16                  # channels per tile (x8 partitions = 128)
NG = CPC // NCH           # channel groups per core
HP = H + 2                # padded rows
VAR = HP * W              # one dj-variant: [130, 128]
IMG = 3 * VAR             # three dj-variants per channel
L = H * W
PPCH = 8                  # partitions per channel
CHF = 18432               # elements per partition per channel (144 chunks)
OUT_CH = 9 * L            # 147456 = PPCH * CHF

# Per-group sub-tile chunk widths (must each be a multiple of 9 so the
# stride-9 multiply stays phase-free, and sum to 144).  Tapered: small
# sub-tiles at the pipeline head (first mul starts sooner) and tail
# (short drain after the last loads).
SCHED = [
    [18, 18, 18, 18, 18, 18, 18, 18],
    [18, 18, 18, 18, 18, 18, 18, 18],
]
assert all(sum(s) == 144 and all(w * 128 % 9 == 0 for w in s) for s in SCHED)
MAXW = max(w for s in SCHED for w in s)
MAXF = MAXW * 128         # largest sub-tile free width per fused half


def _segments(sched):
    """Per sub-tile: (FREE, f_base, [(rr, f_off, len, src_off), ...]) --
    maximal q-runs inside the sub-tile window not crossing partition
    (144) or patch (128) boundaries.  f_base = chunk offset within CHF."""
    starts = [0]
    for w in sched:
        starts.append(starts[-1] + w)
    tiles = []
    for t, w in enumerate(sched):
        s0 = starts[t]
        bounds = sorted(
            {q for q in range(0, 1153, 144)}
            | {q for q in range(0, 1153, 128)}
            | {144 * r + s for r in range(8) for s in (s0, s0 + w)}
        )
        segs = []
        for qs, qe in zip(bounds[:-1], bounds[1:]):
            rr = qs // 144
            s = qs - 144 * rr
            if not (s0 <= s < s0 + w):
                continue
            p2 = qs // 128
            di, dj = divmod(p2, 3)
            segs.append(
                (rr, (s - s0) * 128, (qe - qs) * 128,
                 dj * VAR + (qs - 128 * p2 + di) * W)
            )
        tiles.append((w * 128, s0 * 128, segs))
    return tiles


_TILES = [_segments(s) for s in SCHED]


def _build_program():
    nc = bass.Bass(trn_type="TRN2")
    # k and q fused on a leading [2] axis (and o1/o2 likewise) so one
    # dma_start covers both: 32-descriptor loads / 256-descriptor stores
    # halve the dma_start count and keep all 16 SDMA engines 2 deep.
    kq = nc.dram_tensor("kq", [2, CPC, 3, HP, W], F16, kind="ExternalInput")
    oo = nc.dram_tensor("oo", [2, CPC, OUT_CH], F16, kind="ExternalOutput")
    IN_X = CPC * IMG          # DRAM stride between k and q planes
    OUT_X = CPC * OUT_CH      # DRAM stride between o1 and o2 planes

    # Three dynamic DMA queues (SP-HWDGE, ACT-HWDGE, Pool-SWDGE); strict
    # round-robin keeps every queue fed (prior HW finding: greedy
    # bin-packing clusters DMAs per queue and the per-engine FIFO then
    # serializes them).
    engines = [nc.sync, nc.scalar, nc.gpsimd]
    eng_i = [0]

    def eng():
        e = engines[eng_i[0] % len(engines)]
        eng_i[0] += 1
        return e

    MF2 = 2 * MAXF  # allocated fused tile free width: k then q (o1 then o2)

    def do_loads(g, free, segs, ti):
        # 32 descriptors (channel-major x {k,q}) per dma_start, mutually
        # non-contiguous in stream order.  Descriptors are dealt to
        # SDMA-engine slots round-robin from slot 0 and consecutive
        # contiguous descriptors re-aggregate into one packet, so
        # 8-descriptor loads pile onto engines 0-7 (HW-measured: 86%
        # busy vs 39% on engines 8-15); 32 descriptors keep all 16
        # engines 2 deep.
        th = ti[:].tensor
        for rr, f_off, seg_len, src_off in segs:
            eng().dma_start(
                AP(th, rr * MF2 + f_off,
                   [[PPCH * MF2, NCH], [free, 2], [1, seg_len]]),
                AP(kq, g * NCH * IMG + src_off,
                   [[IMG, NCH], [IN_X, 2], [1, seg_len]]),
            )

    def do_cen(free, ti, cen):
        # Expand the stride-9 centers into a DENSE tile (plane 0 =
        # q-centers for o1, plane 1 = k-centers for o2) on the otherwise
        # idle ACT/GpSimd engines.  The DVE multiplies then have all
        # operands packed step-1 fp16 -> 2x perf mode (HW packs two
        # 16-bit elements per 32-bit read port; the 0-stride broadcast
        # operand of the fused form forces 1x).
        ith, cenh = ti[:].tensor, cen[:].tensor
        n9 = free // 9
        nc.scalar.activation(
            AP(cenh, 0, [[MF2, 128], [9, n9], [1, 9]]),
            AP(ith, free + 4, [[MF2, 128], [9, n9], [0, 9]]),
            mybir.ActivationFunctionType.Copy,
        )
        nc.scalar.activation(
            AP(cenh, free, [[MF2, 128], [9, n9], [1, 9]]),
            AP(ith, 4, [[MF2, 128], [9, n9], [0, 9]]),
            mybir.ActivationFunctionType.Copy,
        )

    def do_mul_store(g, free, f_base, ti, cen, ot):
        ith, cenh, oth = ti[:].tensor, cen[:].tensor, ot[:].tensor
        ap = [[MF2, 128], [1, free]]
        # o1 = k_full * q_center ; o2 = q_full * k_center.  Each output's
        # store issues right after its own multiply so the second half of
        # the store traffic isn't gated on both muls (shorter drain).
        # DRAM per channel is contiguous: partition r = 8*ch + rr maps
        # to offset r*CHF + f_base, uniform across all 128 partitions.
        for x in (0, 1):
            nc.vector.tensor_mul(
                AP(oth, x * free, ap),
                AP(ith, x * free, ap),
                AP(cenh, x * free, ap),
            )
            eng().dma_start(
                AP(oo, x * OUT_X + g * NCH * OUT_CH + f_base,
                   [[CHF, 128], [1, free]]),
                AP(oth, x * free, [[MF2, 128], [1, free]]),
            )

    with tile.TileContext(nc) as tc:
        with (
            tc.tile_pool(name="tin", bufs=4) as tin,
            tc.tile_pool(name="tcen", bufs=3) as tcen,
            tc.tile_pool(name="tout", bufs=3) as tout,
        ):
            # Two-stage lookahead: loads(n) | center-copies(n-1) |
            # muls+stores(n-2).  DMA-queue FIFOs see all loads ahead of
            # the mul-gated stores, and the ACT/GpSimd center-copies
            # (which wait on loads n-1) never head-of-line-block the
            # dma_starts they issue for tile n.
            stage = []
            for g in range(NG):
                for free, f_base, segs in _TILES[g]:
                    ti = tin.tile([128, MF2], F16, tag="ti")
                    do_loads(g, free, segs, ti)
                    cen = tcen.tile([128, MF2], F16, tag="cen")
                    ot = tout.tile([128, MF2], F16, tag="ot")
                    stage.append((g, free, f_base, ti, cen, ot))
                    if len(stage) >= 2:
                        do_cen(stage[-2][1], stage[-2][3], stage[-2][4])
                    if len(stage) >= 3:
                        do_mul_store(*stage[-3])
            do_cen(stage[-1][1], stage[-1][3], stage[-1][4])
            do_mul_store(*stage[-2])
            do_mul_store(*stage[-1])
    _split_waits(nc)
    return nc


_NC_CACHE = []


def _get_nc():
    if not _NC_CACHE:
        _NC_CACHE.append(_build_program())
    return _NC_CACHE[0]


def _variants(x):
    """[B,C,H,W] -> [BC, 3, HP, W] fp16: dj-shifted, row-padded column
    windows of the zero-padded image."""
    xpad = np.pad(
        np.ascontiguousarray(x, dtype=np.float32).reshape(BC, H, W),
        ((0, 0), (1, 1), (1, 1)),
    )
    v = np.stack([xpad[:, :, j : j + W] for j in range(3)], axis=1)
    return np.ascontiguousarray(v.astype(np.float16))


def make_in_maps(key_map, query_map):
    kv = _variants(key_map)
    qv = _variants(query_map)
    return [
        {
            "kq": np.ascontiguousarray(
                np.stack([kv[m * CPC : (m + 1) * CPC],
                          qv[m * CPC : (m + 1) * CPC]])
            ),
        }
        for m in range(N_CORES)
    ]


def assemble(results):
    out1 = np.concatenate([results[m]["oo"][0] for m in range(N_CORES)], axis=0)
    out2 = np.concatenate([results[m]["oo"][1] for m in range(N_CORES)], axis=0)
    return (
        out1.reshape(B, C, L, 9).astype(np.float32),
        out2.reshape(B, C, L, 9).astype(np.float32),
    )


def kernel(key_map, query_map):
    nc = _get_nc()
    in_maps = make_in_maps(key_map, query_map)
    res = run_bass_kernel_spmd(nc, in_maps, core_ids=list(range(N_CORES)))
    return assemble(res.results)


# revision 25
# speedup vs baseline: 1.8454x; 1.0316x over previous
"""Trainium2 Bass kernel for the sparse-attention (local 3x3 unfold) problem.

Math (per batch-channel (b,c), H=W=128, K=3, pad=1):
  ku = unfold(key)  -> [9, L] raw-flat, reinterpreted [L, 9]
  qu = unfold(query)
  out1 = ku * qu[:, 4:5] ; out2 = ku[:, 4:5] * qu   (as [L, 9] views)

The flat per-channel output index n in [0, 9L) decomposes two ways:
  * n = 128*q + j           (chunk q = one (patch p2=q//128, row i2=q%128)
                             slice: 128 contiguous floats of a dj-shifted,
                             row-padded image variant)
  * n = 9*g + e             (group g shares one stride-9 "center" factor)

Device layout (v2, "fat rows"): channel ch of a tile owns 16 partitions
(r = 16*ch + rr) with FREE = 9216 = 72 chunks per partition, n = 9216*rr + f.
  * FREE % 9 == 0 keeps the stride-9 center-broadcast multiply phase-free
    on every partition (one DVE op covers all 8 channels of a tile).
  * Loads: the (72-chunk partition) x (128-chunk patch) overlap gives 24
    maximal segments per channel; each is ONE contiguous DRAM run of the
    variant image -> one descriptor (2-18 KiB) per (segment, channel).
  * Stores: per-channel DRAM is contiguous with offset r*FREE uniform in
    the partition index -> one dma_start moves a whole tile half
    (128 descriptors x 9 KiB).

dtype: fp16 end-to-end on device (harness tolerance 2e-2 vs ~1.5e-3 fp16
error); host upcasts to fp32.  Halves both HBM read and write traffic.

Sharding: pure data-parallel over the 256 (b,c) channels; 32 per core.
"""

import sys

for _p in ("/opt/trn_rl_repo", "/opt/pypackages"):
    if _p not in sys.path:
        sys.path.insert(0, _p)

import numpy as np

import concourse.bass as bass
import concourse.mybir as mybir
import concourse.tile as tile
from concourse.bass import AP
from concourse.bass_utils import run_bass_kernel_spmd
from concourse.vector_clock import ScopedClock

# ---------------------------------------------------------------------------
# Patch: this container's walrus rejects >1 sync-wait on the Tile tail Drain
# ("Too many sync wait commands").  Spill extra waits onto SP NOPs, which
# execute in program order before the all-engine barrier, preserving the
# "all work done before sem clear" semantics.
# ---------------------------------------------------------------------------


def _drain_and_barrier(self, tick_clock, wait_clock):
    nc = self.nc
    drain_inst = nc.sync.drain()
    wait_clock.add_sem_waits(
        drain_inst.ins, ScopedClock({None: tick_clock.global_clock})
    )
    si = drain_inst.ins.sync_info
    if si is not None and len(si.on_wait) > 1:
        waits = list(si.on_wait)
        drain_inst.ins.sync_info = mybir.SyncInfo(
            on_wait=waits[:1], on_update=list(si.on_update)
        )
        for w in waits[1:]:
            nop = nc.sync.nop(nofuse=True)
            nop.ins.sync_info = mybir.SyncInfo(on_wait=[w], on_update=[])

    nc.all_engine_barrier()
    assert self.sems is not None
    popped = nc._tile_sem_poison_stack.pop()
    assert popped is self._sem_poison
    nc.clear_and_free_semaphores(list(self.sems.allocated().values()))
    nc.all_engine_barrier()


tile.TileContext._drain_and_barrier = _drain_and_barrier


def _split_waits(nc, maxw=1):
    """Walrus here allows only `maxw` sync-waits per instruction: move extra
    waits onto same-engine NOPs inserted immediately before the instruction
    (same engine stream => executes before it)."""
    for fn in nc.m.functions:
        for bb in fn.blocks:
            out = []
            for inst in bb.instructions:
                si = getattr(inst, "sync_info", None)
                if si is not None and len(si.on_wait) > maxw:
                    waits = list(si.on_wait)
                    for w in waits[:-maxw]:
                        nop = mybir.InstNoOp(
                            name=nc.get_next_instruction_name(),
                            bass_nofuse=True,
                        )
                        nop.engine = inst.engine
                        nop.sync_info = mybir.SyncInfo(on_wait=[w], on_update=[])
                        nc.register_instruction(nop)
                        out.append(nop)
                    inst.sync_info = mybir.SyncInfo(
                        on_wait=waits[-maxw:], on_update=list(si.on_update)
                    )
                out.append(inst)
            bb.instructions[:] = out

# ---------------------------------------------------------------------------

F16 = mybir.dt.float16

N_CORES = 8
B, C, H, W = 4, 64, 128, 128
BC = B * C                # 256 channels
CPC = BC // N_CORES       # 32 channels per core
NCH = 16                  # channels per tile (x8 partitions = 128)
NG = CPC // NCH           # channel groups per core
HP = H + 2                # padded rows
VAR = HP * W              # one dj-variant: [130, 128]
IMG = 3 * VAR             # three dj-variants per channel
L = H * W
PPCH = 8                  # partitions per channel
CHF = 18432               # elements per partition per channel (144 chunks)
OUT_CH = 9 * L            # 147456 = PPCH * CHF

# Per-group sub-tile chunk widths (must each be a multiple of 9 so the
# stride-9 multiply stays phase-free, and sum to 144).  Tapered: small
# sub-tiles at the pipeline head (first mul starts sooner) and tail
# (short drain after the last loads).
SCHED = [
    [18, 18, 18, 18, 18, 18, 18, 18],
    [18, 18, 18, 18, 18, 18, 18, 18],
]
assert all(sum(s) == 144 and all(w * 128 % 9 == 0 for w in s) for s in SCHED)
MAXW = max(w for s in SCHED for w in s)
MAXF = MAXW * 128         # largest sub-tile free width per fused half


def _segments(sched):
    """Per sub-tile: (FREE, f_base, [(rr, f_off, len, src_off), ...]) --
    maximal q-runs inside the sub-tile window not crossing partition
    (144) or patch (128) boundaries.  f_base = chunk offset within CHF."""
    starts = [0]
    for w in sched:
        starts.append(starts[-1] + w)
    tiles = []
    for t, w in enumerate(sched):
        s0 = starts[t]
        bounds = sorted(
            {q for q in range(0, 1153, 144)}
            | {q for q in range(0, 1153, 128)}
            | {144 * r + s for r in range(8) for s in (s0, s0 + w)}
        )
        segs = []
        for qs, qe in zip(bounds[:-1], bounds[1:]):
            rr = qs // 144
            s = qs - 144 * rr
            if not (s0 <= s < s0 + w):
                continue
            p2 = qs // 128
            di, dj = divmod(p2, 3)
            segs.append(
                (rr, (s - s0) * 128, (qe - qs) * 128,
                 dj * VAR + (qs - 128 * p2 + di) * W)
            )
        tiles.append((w * 128, s0 * 128, segs))
    return tiles


_TILES = [_segments(s) for s in SCHED]


def _build_program():
    nc = bass.Bass(trn_type="TRN2")
    # k and q fused on a leading [2] axis (and o1/o2 likewise) so one
    # dma_start covers both: 32-descriptor loads / 256-descriptor stores
    # halve the dma_start count and keep all 16 SDMA engines 2 deep.
    kq = nc.dram_tensor("kq", [2, CPC, 3, HP, W], F16, kind="ExternalInput")
    oo = nc.dram_tensor("oo", [2, CPC, OUT_CH], F16, kind="ExternalOutput")
    IN_X = CPC * IMG          # DRAM stride between k and q planes
    OUT_X = CPC * OUT_CH      # DRAM stride between o1 and o2 planes

    # Three dynamic DMA queues (SP-HWDGE, ACT-HWDGE, Pool-SWDGE); strict
    # round-robin keeps every queue fed (prior HW finding: greedy
    # bin-packing clusters DMAs per queue and the per-engine FIFO then
    # serializes them).
    engines = [nc.sync, nc.scalar, nc.gpsimd]
    eng_i = [0]

    def eng():
        e = engines[eng_i[0] % len(engines)]
        eng_i[0] += 1
        return e

    MF2 = 2 * MAXF  # allocated fused tile free width: k then q (o1 then o2)

    def do_loads(g, free, segs, ti):
        # 32 descriptors (channel-major x {k,q}) per dma_start, mutually
        # non-contiguous in stream order.  Descriptors are dealt to
        # SDMA-engine slots round-robin from slot 0 and consecutive
        # contiguous descriptors re-aggregate into one packet, so
        # 8-descriptor loads pile onto engines 0-7 (HW-measured: 86%
        # busy vs 39% on engines 8-15); 32 descriptors keep all 16
        # engines 2 deep.
        th = ti[:].tensor
        for rr, f_off, seg_len, src_off in segs:
            eng().dma_start(
                AP(th, rr * MF2 + f_off,
                   [[PPCH * MF2, NCH], [free, 2], [1, seg_len]]),
                AP(kq, g * NCH * IMG + src_off,
                   [[IMG, NCH], [IN_X, 2], [1, seg_len]]),
            )

    def do_mul_store(g, free, f_base, ti, ot):
        ith, oth = ti[:].tensor, ot[:].tensor
        ap_o = [[MF2, 128], [9, free // 9], [1, 9]]
        ap_b = [[MF2, 128], [9, free // 9], [0, 9]]
        # o1 = k_full * q_center ; o2 = q_full * k_center.  Each output's
        # store issues right after its own multiply so the second half of
        # the store traffic isn't gated on both muls (shorter drain).
        # (Dense-center rewrites that unlock the DVE 2x perf mode were
        # HW-measured net-neutral: the ACT/GpSimd expansion copies cost
        # more than the DVE multiply saves, and DMA stays the bottleneck.)
        # DRAM per channel is contiguous: partition r = 8*ch + rr maps
        # to offset r*CHF + f_base, uniform across all 128 partitions.
        for x in (0, 1):
            nc.vector.tensor_mul(
                AP(oth, x * free, ap_o),
                AP(ith, x * free, ap_o),
                AP(ith, (1 - x) * free + 4, ap_b),
            )
            eng().dma_start(
                AP(oo, x * OUT_X + g * NCH * OUT_CH + f_base,
                   [[CHF, 128], [1, free]]),
                AP(oth, x * free, [[MF2, 128], [1, free]]),
            )

    with tile.TileContext(nc) as tc:
        with (
            tc.tile_pool(name="tin", bufs=3) as tin,
            tc.tile_pool(name="tout", bufs=3) as tout,
        ):
            # Software pipeline with one-tile lookahead so loads of tile
            # n+1 sit AHEAD of (mul-gated) stores of tile n in each DMA
            # engine's FIFO -> no head-of-line blocking on the loads.
            prev = None
            for g in range(NG):
                for free, f_base, segs in _TILES[g]:
                    ti = tin.tile([128, MF2], F16, tag="ti")
                    do_loads(g, free, segs, ti)
                    if prev is not None:
                        do_mul_store(*prev)
                    ot = tout.tile([128, MF2], F16, tag="ot")
                    prev = (g, free, f_base, ti, ot)
            do_mul_store(*prev)
    _split_waits(nc)
    return nc


_NC_CACHE = []


def _get_nc():
    if not _NC_CACHE:
        _NC_CACHE.append(_build_program())
    return _NC_CACHE[0]


def _variants(x):
    """[B,C,H,W] -> [BC, 3, HP, W] fp16: dj-shifted, row-padded column
    windows of the zero-padded image."""
    xpad = np.pad(
        np.ascontiguousarray(x, dtype=np.float32).reshape(BC, H, W),
        ((0, 0), (1, 1), (1, 1)),
    )
    v = np.stack([xpad[:, :, j : j + W] for j in range(3)], axis=1)
    return np.ascontiguousarray(v.astype(np.float16))


def make_in_maps(key_map, query_map):
    kv = _variants(key_map)
    qv = _variants(query_map)
    return [
        {
            "kq": np.ascontiguousarray(
                np.stack([kv[m * CPC : (m + 1) * CPC],
                          qv[m * CPC : (m + 1) * CPC]])
            ),
        }
        for m in range(N_CORES)
    ]


def assemble(results):
    out1 = np.concatenate([results[m]["oo"][0] for m in range(N_CORES)], axis=0)
    out2 = np.concatenate([results[m]["oo"][1] for m in range(N_CORES)], axis=0)
    return (
        out1.reshape(B, C, L, 9).astype(np.float32),
        out2.reshape(B, C, L, 9).astype(np.float32),
    )


def kernel(key_map, query_map):
    nc = _get_nc()
    in_maps = make_in_maps(key_map, query_map)
    res = run_bass_kernel_spmd(nc, in_maps, core_ids=list(range(N_CORES)))
    return assemble(res.results)
